# revision 12
# baseline (speedup 1.0000x reference)
"""Trainium2 Bass kernel for nn_DeepFM_55439437857626.

Strategy (8 NeuronCores, SPMD):
  * Data-parallel over batch: 16384 rows -> 2048 per core.
  * Embedding lookup on-device: all five embedding tables plus the matching
    fm_linear weight columns are packed host-side into one combined table
    [1000074, 66] ( [emb(64) | fm_w.T(2)] per row ), replicated to every
    core's HBM.  Each 128-row batch tile does ONE gpsimd indirect DMA with a
    [128, 5] index tile, gathering all 5 field rows per batch element.
  * FM first/second order computed in batch-on-partition layout.
  * DNN: activations transposed to feature-major via PE transposes; matmuls
    run as float32r (full PE rate) over 512-wide batch groups; `visual` is
    host-transposed so its tiles DMA as clean [128, 512] slices.
  * Final logits + 2-way softmax on-chip; per-core [2048, 2] outputs are
    concatenated on host.
"""

import os
import sys

sys.path.insert(0, "/opt/trn_rl_repo")

import numpy as np

import concourse.bacc as bacc
import concourse.bass as bass
import concourse.tile as tile
from concourse import mybir
from concourse.bass_utils import run_bass_kernel_spmd
from concourse.masks import make_identity

# ---- problem constants (hardcoded per contract) ----
NUM_USERS = 1_000_000
N_HOUR, N_GENDER, N_AGE, N_ATTR = 24, 2, 8, 40
VOCAB = NUM_USERS + N_HOUR + N_GENDER + N_AGE + N_ATTR  # 1000074
K = 64          # FACE_K
VIS = 2048      # VISUAL_DIM
HID = 512
B = 16384
NCORES = 8
BC = B // NCORES          # 2048 per core
P = 128                   # partitions / batch tile
NT = BC // P              # 16 batch tiles per core
GT = 4                    # tiles per matmul group (512 batch)
NG = NT // GT             # 4 groups

F32 = mybir.dt.float32
BF16 = mybir.dt.bfloat16
I32 = mybir.dt.int32


def build_nc():
    nc = bacc.Bacc(trn_type="TRN2")

    table = nc.dram_tensor("table", [VOCAB, 66], F32, kind="ExternalInput")
    cols = nc.dram_tensor("cols", [BC, 5], I32, kind="ExternalInput")
    vt = nc.dram_tensor("vt", [VIS, BC], BF16, kind="ExternalInput")
    scale = nc.dram_tensor("scale", [BC, 1], F32, kind="ExternalInput")
    w1t = nc.dram_tensor("w1t", [P, 3, HID], BF16, kind="ExternalInput")
    w2t = nc.dram_tensor("w2t", [P, 4, HID], BF16, kind="ExternalInput")
    w3t = nc.dram_tensor("w3t", [P, 4, 2], BF16, kind="ExternalInput")
    vwt = nc.dram_tensor("vwt", [P, 16, K], BF16, kind="ExternalInput")
    b1 = nc.dram_tensor("b1", [P, 4], F32, kind="ExternalInput")
    b2 = nc.dram_tensor("b2", [P, 4], F32, kind="ExternalInput")
    vb = nc.dram_tensor("vb", [K, 1], F32, kind="ExternalInput")
    bout = nc.dram_tensor("bout", [1, 2], F32, kind="ExternalInput")
    out = nc.dram_tensor("out", [BC, 2], F32, kind="ExternalOutput")

    with tile.TileContext(nc) as tc:
        with (
            tc.tile_pool(name="singles", bufs=1) as singles,
            tc.tile_pool(name="gtiles", bufs=2 * GT) as gpool,
            tc.tile_pool(name="scratch", bufs=2 * GT) as spool,
            tc.tile_pool(name="xg", bufs=2) as xgpool,
            tc.tile_pool(name="hs", bufs=2) as hpool,
            tc.tile_pool(name="vload", bufs=4) as vpool,
            tc.tile_pool(name="fin", bufs=2 * GT) as fpool,
            tc.tile_pool(name="ps_mm", bufs=4, space="PSUM") as ps_mm,
            tc.tile_pool(name="ps_tr", bufs=2, space="PSUM") as ps_tr,
            tc.tile_pool(name="ps_vis", bufs=1, space="PSUM") as ps_vis,
            tc.tile_pool(name="ps_h3", bufs=1, space="PSUM") as ps_h3,
        ):
            # ---- load constants / weights once ----
            ident = singles.tile([P, P], F32)
            make_identity(nc, ident[:])

            w1t_s = singles.tile([P, 3, HID], BF16)
            nc.sync.dma_start(out=w1t_s[:], in_=w1t[:, :, :])
            w2t_s = singles.tile([P, 4, HID], BF16)
            nc.sync.dma_start(out=w2t_s[:], in_=w2t[:, :, :])
            w3t_s = singles.tile([P, 4, 2], BF16)
            nc.sync.dma_start(out=w3t_s[:], in_=w3t[:, :, :])
            vwt_s = singles.tile([P, 16, K], BF16)
            nc.sync.dma_start(out=vwt_s[:], in_=vwt[:, :, :])
            b1_s = singles.tile([P, 4], F32)
            nc.sync.dma_start(out=b1_s[:], in_=b1[:, :])
            b2_s = singles.tile([P, 4], F32)
            nc.sync.dma_start(out=b2_s[:], in_=b2[:, :])
            vb_s = singles.tile([K, 1], F32)
            nc.sync.dma_start(out=vb_s[:], in_=vb[:, :])
            # fm_b + b3 (host-combined), broadcast to all partitions
            bias2_s = singles.tile([P, 2], F32)
            nc.sync.dma_start(out=bias2_s[:], in_=bout[0:1, :].to_broadcast([P, 2]))

            for g in range(NG):
                uh_l, ft_l, lp_l, so_l = [], [], [], []
                # ---------- phase A: gather + FM (batch-on-partition) ----------
                for t in range(GT):
                    T = g * GT + t
                    r0 = T * P
                    idx = gpool.tile([P, 5], I32, tag="idx")
                    nc.sync.dma_start(out=idx[:], in_=cols[r0 : r0 + P, :])
                    sc = gpool.tile([P, 1], F32, tag="sc")
                    nc.sync.dma_start(out=sc[:], in_=scale[r0 : r0 + P, :])

                    gath = gpool.tile([P, 5, 66], F32, tag="gath")
                    # NOTE: multi-index-per-partition offset APs diverge from
                    # the sim on HW; one [P,1]-indexed gather per field is the
                    # validated shape.
                    for j in range(5):
                        nc.gpsimd.indirect_dma_start(
                            out=gath[:, j, :],
                            out_offset=None,
                            in_=table[:, :],
                            in_offset=bass.IndirectOffsetOnAxis(
                                ap=idx[:, j : j + 1], axis=0
                            ),
                        )

                    # user/hour embeddings packed contiguously [128, 2, 64]
                    uh = gpool.tile([P, 2, K], F32, tag="uh")
                    nc.vector.tensor_copy(out=uh[:], in_=gath[:, 0:2, 0:K])
                    # face embeddings: scale * tanh(e)
                    ft = gpool.tile([P, 3, K], F32, tag="ft")
                    nc.scalar.activation(
                        out=ft[:], in_=gath[:, 2:5, 0:K],
                        func=mybir.ActivationFunctionType.Tanh,
                    )
                    nc.vector.tensor_scalar_mul(out=ft[:], in0=ft[:], scalar1=sc[:])

                    # sum of the 5 embeddings
                    esum = spool.tile([P, K], F32, tag="esum")
                    nc.vector.tensor_add(out=esum[:], in0=uh[:, 0, :], in1=uh[:, 1, :])
                    nc.vector.tensor_add(out=esum[:], in0=esum[:], in1=ft[:, 0, :])
                    nc.vector.tensor_add(out=esum[:], in0=esum[:], in1=ft[:, 1, :])
                    nc.vector.tensor_add(out=esum[:], in0=esum[:], in1=ft[:, 2, :])

                    # sum of squares via ACT Square + accumulate
                    sq = spool.tile([P, 3, K], F32, tag="sq")
                    a_uh = spool.tile([P, 1], F32, tag="a_uh")
                    nc.scalar.activation(
                        out=sq[:, 0:2, :], in_=uh[:],
                        func=mybir.ActivationFunctionType.Square,
                        accum_out=a_uh[:],
                    )
                    a_ft = spool.tile([P, 1], F32, tag="a_ft")
                    nc.scalar.activation(
                        out=sq[:], in_=ft[:],
                        func=mybir.ActivationFunctionType.Square,
                        accum_out=a_ft[:],
                    )
                    a_es = spool.tile([P, 1], F32, tag="a_es")
                    nc.scalar.activation(
                        out=sq[:, 0, :], in_=esum[:],
                        func=mybir.ActivationFunctionType.Square,
                        accum_out=a_es[:],
                    )
                    so = gpool.tile([P, 1], F32, tag="so")
                    nc.vector.tensor_tensor(
                        out=so[:], in0=a_es[:], in1=a_uh[:],
                        op=mybir.AluOpType.subtract,
                    )
                    nc.vector.tensor_tensor(
                        out=so[:], in0=so[:], in1=a_ft[:],
                        op=mybir.AluOpType.subtract,
                    )
                    nc.vector.tensor_scalar_mul(out=so[:], in0=so[:], scalar1=0.5)

                    # fm linear: sum the 5 gathered weight column pairs
                    lp = gpool.tile([P, 2], F32, tag="lp")
                    nc.vector.reduce_sum(
                        out=lp[:],
                        in_=gath[:, :, 64:66].transpose([0, 2, 1]),
                        axis=mybir.AxisListType.X,
                    )

                    uh_l.append(uh); ft_l.append(ft); lp_l.append(lp); so_l.append(so)

                # ---------- phase B: transposes -> feature-major x ----------
                xg0 = xgpool.tile([P, 512], BF16, tag="xg0")  # [user|hour]^T
                xg1 = xgpool.tile([P, 512], BF16, tag="xg1")  # [gender'|attr']^T
                xg2 = xgpool.tile([P, 512], BF16, tag="xg2")  # [age' | vis]^T
                for t in range(GT):
                    cs = slice(t * P, (t + 1) * P)
                    tr0 = ps_tr.tile([P, P], F32, tag="tr")
                    nc.tensor.transpose(out=tr0[:], in_=uh_l[t][:], identity=ident[:])
                    nc.vector.tensor_copy(out=xg0[:, cs], in_=tr0[:])
                    tr1 = ps_tr.tile([P, P], F32, tag="tr")
                    nc.tensor.transpose(
                        out=tr1[:], in_=ft_l[t][:, 0:2, :], identity=ident[:]
                    )
                    nc.vector.tensor_copy(out=xg1[:, cs], in_=tr1[:])
                    tr2 = ps_tr.tile([P, P], F32, tag="tr")
                    nc.tensor.transpose(
                        out=tr2[:64, :], in_=ft_l[t][:, 2, :], identity=ident[:]
                    )
                    nc.vector.tensor_copy(out=xg2[:64, cs], in_=tr2[:64, :])

                # ---------- vis = visu_w @ visual^T (feature-major) ----------
                vis_ps = ps_vis.tile([K, 512], F32, tag="vis")
                for k in range(16):
                    vtile = vpool.tile([P, 512], BF16, tag="vtile")
                    nc.sync.dma_start(
                        out=vtile[:],
                        in_=vt[k * P : (k + 1) * P, g * 512 : (g + 1) * 512],
                    )
                    nc.tensor.matmul(
                        out=vis_ps[:],
                        lhsT=(vwt_s[:, k, :]),
                        rhs=(vtile[:]),
                        start=(k == 0),
                        stop=(k == 15),
                    )
                nc.scalar.activation(
                    out=xg2[64:128, :], in_=vis_ps[:],
                    func=mybir.ActivationFunctionType.Identity,
                    bias=vb_s[:, 0:1],
                )

                # ---------- h1 = relu(w1 @ x + b1), feature-major ----------
                xgs = [xg0, xg1, xg2]
                h1t = hpool.tile([P, 4, HID], BF16, tag="h1t")
                for m in range(4):
                    mm = ps_mm.tile([P, 512], F32, tag="mm")
                    for k in range(3):
                        nc.tensor.matmul(
                            out=mm[:],
                            lhsT=(w1t_s[:, k, m * P : (m + 1) * P]),
                            rhs=(xgs[k][:]),
                            start=(k == 0),
                            stop=(k == 2),
                        )
                    nc.scalar.activation(
                        out=h1t[:, m, :], in_=mm[:],
                        func=mybir.ActivationFunctionType.Relu,
                        bias=b1_s[:, m : m + 1],
                    )

                # ---------- h2 = relu(w2 @ h1 + b2) ----------
                h2t = hpool.tile([P, 4, HID], BF16, tag="h2t")
                for m in range(4):
                    mm = ps_mm.tile([P, 512], F32, tag="mm")
                    for k in range(4):
                        nc.tensor.matmul(
                            out=mm[:],
                            lhsT=(w2t_s[:, k, m * P : (m + 1) * P]),
                            rhs=(h1t[:, k, :]),
                            start=(k == 0),
                            stop=(k == 3),
                        )
                    nc.scalar.activation(
                        out=h2t[:, m, :], in_=mm[:],
                        func=mybir.ActivationFunctionType.Relu,
                        bias=b2_s[:, m : m + 1],
                    )

                # ---------- h3 (back to batch-on-partition) + softmax ----------
                for t in range(GT):
                    T = g * GT + t
                    cs = slice(t * P, (t + 1) * P)
                    h3 = ps_h3.tile([P, 2], F32, tag="h3")
                    for k in range(4):
                        nc.tensor.matmul(
                            out=h3[:],
                            lhsT=h2t[:, k, cs],
                            rhs=w3t_s[:, k, :],
                            start=(k == 0),
                            stop=(k == 3),
                        )
                    logits = fpool.tile([P, 2], F32, tag="logits")
                    nc.vector.tensor_add(out=logits[:], in0=h3[:], in1=lp_l[t][:])
                    nc.vector.tensor_add(out=logits[:], in0=logits[:], in1=bias2_s[:])
                    nc.vector.tensor_scalar_add(
                        out=logits[:], in0=logits[:], scalar1=so_l[t][:]
                    )
                    mx = fpool.tile([P, 1], F32, tag="mx")
                    nc.vector.reduce_max(
                        out=mx[:], in_=logits[:], axis=mybir.AxisListType.X
                    )
                    nc.vector.tensor_scalar_sub(
                        out=logits[:], in0=logits[:], scalar1=mx[:]
                    )
                    pr = fpool.tile([P, 2], F32, tag="pr")
                    ssum = fpool.tile([P, 1], F32, tag="ssum")
                    nc.scalar.activation(
                        out=pr[:], in_=logits[:],
                        func=mybir.ActivationFunctionType.Exp,
                        accum_out=ssum[:],
                    )
                    rcp = fpool.tile([P, 1], F32, tag="rcp")
                    nc.vector.reciprocal(out=rcp[:], in_=ssum[:])
                    nc.vector.tensor_scalar_mul(out=pr[:], in0=pr[:], scalar1=rcp[:])
                    nc.sync.dma_start(out=out[T * P : (T + 1) * P, :], in_=pr[:])

    nc.compile()
    return nc


def prep_inputs(inputs):
    """Host-side layout prep: pack tables/weights, transpose visual, slice."""
    f32 = np.float32
    user_emb = np.asarray(inputs["user_emb"], f32)
    hour_emb = np.asarray(inputs["hour_emb"], f32)
    gender_emb = np.asarray(inputs["gender_emb"], f32)
    age_emb = np.asarray(inputs["age_emb"], f32)
    attr_emb = np.asarray(inputs["attr_emb"], f32)
    fm_w = np.asarray(inputs["fm_w"], f32)

    table = np.empty((VOCAB, 66), f32)
    o = 0
    for emb in (user_emb, hour_emb, gender_emb, age_emb, attr_emb):
        table[o : o + emb.shape[0], 0:K] = emb
        o += emb.shape[0]
    assert o == VOCAB
    table[:, 64:66] = fm_w.T

    user_id = np.asarray(inputs["user_id"]).astype(np.int64)
    hour = np.asarray(inputs["hour"]).astype(np.int64)
    gender = np.asarray(inputs["gender"]).astype(np.int64)
    age = np.asarray(inputs["age"]).astype(np.int64)
    attribute = np.asarray(inputs["attribute"]).astype(np.int64)
    OFF_H = NUM_USERS
    OFF_G = NUM_USERS + 24
    OFF_AGE = OFF_G + 2
    OFF_ATTR = OFF_AGE + 8
    # field order must match x-vector layout: user, hour, gender, attr, age
    cols = np.stack(
        [user_id, OFF_H + hour, OFF_G + gender, OFF_ATTR + attribute, OFF_AGE + age],
        axis=1,
    ).astype(np.int32)

    bf16 = mybir.dt.np(BF16)
    visual = np.asarray(inputs["visual"], f32)
    vT = np.ascontiguousarray(visual.T).astype(bf16)  # [2048, B]
    scale = np.ascontiguousarray(np.asarray(inputs["scale"], f32))

    w1 = np.asarray(inputs["w1"], f32)  # [512, 384]
    w2 = np.asarray(inputs["w2"], f32)  # [512, 512]
    w3 = np.asarray(inputs["w3"], f32)  # [2, 512]
    visu_w = np.asarray(inputs["visu_w"], f32)  # [64, 2048]
    w1t = np.ascontiguousarray(w1.T.reshape(3, P, HID).transpose(1, 0, 2)).astype(bf16)
    w2t = np.ascontiguousarray(w2.T.reshape(4, P, HID).transpose(1, 0, 2)).astype(bf16)
    w3t = np.ascontiguousarray(w3.T.reshape(4, P, 2).transpose(1, 0, 2)).astype(bf16)
    vwt = np.ascontiguousarray(visu_w.T.reshape(16, P, K).transpose(1, 0, 2)).astype(bf16)
    b1 = np.ascontiguousarray(np.asarray(inputs["b1"], f32).reshape(4, P).T)
    b2 = np.ascontiguousarray(np.asarray(inputs["b2"], f32).reshape(4, P).T)
    vb = np.asarray(inputs["visu_b"], f32).reshape(K, 1)
    bout = (
        np.asarray(inputs["fm_b"], f32) + np.asarray(inputs["b3"], f32)
    ).reshape(1, 2)

    shared = dict(
        table=table, w1t=w1t, w2t=w2t, w3t=w3t, vwt=vwt,
        b1=b1, b2=b2, vb=vb, bout=bout,
    )
    in_maps = []
    for c in range(NCORES):
        s = slice(c * BC, (c + 1) * BC)
        m = dict(shared)
        m["cols"] = np.ascontiguousarray(cols[s])
        m["vt"] = np.ascontiguousarray(vT[:, s])
        m["scale"] = scale[s]
        in_maps.append(m)
    return in_maps


_NC_CACHE = None
LAST_RESULTS = None  # test.py introspection (exec_time_ns when traced)


def kernel(**inputs) -> np.ndarray:
    global _NC_CACHE, LAST_RESULTS
    if _NC_CACHE is None:
        _NC_CACHE = build_nc()
    nc = _NC_CACHE
    in_maps = prep_inputs(inputs)
    res = run_bass_kernel_spmd(nc, in_maps, core_ids=list(range(NCORES)))
    LAST_RESULTS = res
    return np.concatenate([res.results[c]["out"] for c in range(NCORES)], axis=0)
